# revision 10
# baseline (speedup 1.0000x reference)
"""Routed MoE classifier head for Trainium2 (8 NeuronCores, SPMD).

The reference computes all 8 experts densely and selects; here each sample is
routed to exactly one expert.  On the host we gather samples by expert
(expert e -> core e), pad to a common S, and pre-transpose x so the
contraction dim D lands on SBUF partitions.  Each core runs a dense 2-layer
MLP (768 -> relu 384 -> 8) over its expert's samples:

  layer 1:  h^T = relu(W1^T x^T + b1)   as matmul(psum, lhsT=W1 [128,128],
            rhs=xT [128,n]) accumulated over 6 d-blocks per h-block
  layer 2:  y^T = W2^T h^T + b2

Matmul operands use bfloat16 (same 1 col/cycle PE stream rate as f32r, but
half the HBM traffic and no ap<256 stream penalty); PSUM accumulation stays
fp32.  All of x is DMA-preloaded into SBUF up front (it fits in bf16), and
the PE is pre-warmed with dummy matmuls so the HAM clock-gate ramp starts
during the DMA wait.  Output y^T [8, S] is scattered back on the host.
"""

import numpy as np
import ml_dtypes

import concourse.bass as bass
import concourse.mybir as mybir
from concourse.tile import TileContext
from concourse.bass_utils import run_bass_kernel_spmd

P = 128
D = 768
H = 384
C = 8
E = 8
NCORES = 8
DBLK = D // P  # 6
HBLK = H // P  # 3
CHUNK = 512  # compute chunk (one PSUM bank of fp32 — ISA max matmul width)
LEAD = 512  # first chunk (quick start while DMA streams)

BF16NP = ml_dtypes.bfloat16

_program_cache = {}
last_results = None  # BassKernelResults of the most recent run (for test harness)


def _split_excess_waits(nc, max_waits=1):
    """The walrus build in this container only encodes one sem-wait per
    instruction; hoist extra waits onto NOPs inserted just before."""
    for blk in nc.main_func.blocks:
        insts = blk.instructions
        i = 0
        while i < len(insts):
            inst = insts[i]
            si = getattr(inst, "sync_info", None)
            if si is not None and si.on_wait and len(si.on_wait) > max_waits:
                waits = list(si.on_wait)
                extra, keep = waits[:-max_waits], waits[-max_waits:]
                nops = []
                for j in range(0, len(extra), max_waits):
                    nops.append(
                        mybir.InstNoOp(
                            name=f"{inst.name}-wsplit{j}",
                            engine=inst.engine,
                            bass_nofuse=True,
                            sync_info=mybir.SyncInfo(
                                on_wait=extra[j : j + max_waits], on_update=[]
                            ),
                        )
                    )
                inst.sync_info = mybir.SyncInfo(on_wait=keep, on_update=si.on_update)
                for k, nop in enumerate(nops):
                    nc.register_instruction(nop, overwrite=True)
                    insts.insert(i + k, nop)
                i += len(nops)
            i += 1
    return nc


def _chunks_of(S):
    """Chunk widths covering S: CHUNK-wide, then tail."""
    chunks = []
    rem = S
    while rem >= CHUNK:
        chunks.append(CHUNK)
        rem -= CHUNK
    if rem:
        chunks.append(rem)
    return chunks


def _build_program(S):
    f32 = mybir.dt.float32
    bf16 = mybir.dt.bfloat16
    relu = mybir.ActivationFunctionType.Relu
    add = mybir.AluOpType.add

    chunks = _chunks_of(S)

    nc = bass.Bass(enable_partition_id=False)
    xt = nc.dram_tensor("xt", [P, DBLK, S], bf16, kind="ExternalInput")
    # w1 (6*384 cols) and w2 (3*8 cols) packed on the same 128 partitions
    wt = nc.dram_tensor("wt", [P, DBLK * H + HBLK * C], bf16, kind="ExternalInput")
    # b1 (3 cols, per h-block) and b2 (1 col, rows 0..7) packed
    bt = nc.dram_tensor("bt", [P, HBLK + 1], f32, kind="ExternalInput")
    yt = nc.dram_tensor("yt", [C, S], f32, kind="ExternalOutput")

    with TileContext(nc) as tc:
        with (
            tc.tile_pool(name="const", bufs=1) as cpool,
            tc.tile_pool(name="xin", bufs=1) as xpool,
            tc.tile_pool(name="hbuf", bufs=3) as hpool,
            tc.tile_pool(name="yout", bufs=2) as ypool,
            tc.tile_pool(name="psum1", bufs=6, space="PSUM") as pp1,
            tc.tile_pool(name="psum2", bufs=2, space="PSUM") as pp2,
        ):
            # -------- startup: fan DMA dispatches across queues, pre-warm PE
            # PE clock ramp (HAM) needs ~3.4us of activity before it doubles
            # the clock; burn the DMA-wait window on dummy matmuls over a
            # zeroed scratch tile so real matmuls start closer to full speed.
            scratch = cpool.tile([P, 512], bf16)
            nc.gpsimd.memset(scratch[:], 0.0)
            for _ in range(3):
                pw = pp1.tile([P, LEAD], f32, name="ps", tag="ps")
                nc.tensor.matmul(
                    pw[:, :LEAD], scratch[:, :P], scratch[:, :LEAD],
                    start=True, stop=True,
                )

            # weights on the Scalar HWDGE ring in 3 pieces so the first
            # d-block's weights land before the whole stack transfers
            w_t = cpool.tile([P, DBLK * H + HBLK * C], bf16)
            nc.scalar.dma_start(w_t[:, 0:H], wt[:, 0:H])
            nc.scalar.dma_start(w_t[:, H : 2 * H], wt[:, H : 2 * H])
            nc.scalar.dma_start(w_t[:, 2 * H :], wt[:, 2 * H :])

            b_t = cpool.tile([P, HBLK + 1], f32)
            nc.gpsimd.dma_start(b_t[:], bt[:])

            # Warm the ACT table during the startup DMA window so the
            # first real relu doesn't pay the ~1.3us table load.
            warm = cpool.tile([P, 1], f32)
            nc.vector.memset(warm[:], 0.0)
            nc.scalar.activation(warm[:], warm[:], relu, bias=0.0)

            # ALL of x, dispatched up front on the Sync HWDGE ring (12.9MB
            # bf16 fits in SBUF; DMA at ~330GB/s finishes well ahead of the
            # ~74us PE stream).  Chunk 0 goes per-d-block so the very first
            # matmul only waits on a 128KB slice.
            offs = [0]
            for n in chunks:
                offs.append(offs[-1] + n)
            xts = [
                xpool.tile([P, DBLK, n], bf16, name=f"x{ci}", tag=f"x{ci}")
                for ci, n in enumerate(chunks)
            ]

            def load_chunk(eng, ci):
                eng.dma_start(
                    xts[ci][:, :, :], xt[:, :, offs[ci] : offs[ci] + chunks[ci]]
                )

            # chunk 0 per-d-block on sync (first matmul waits only on db0);
            # chunk 1 on the idle gpsimd SWDGE, chunk 2 wedged between chunk
            # 0's d-block dispatches — so neither queues behind the other
            # six sync dispatches the way a strict FIFO order would force.
            for db in range(3):
                nc.sync.dma_start(xts[0][:, db, :], xt[:, db, 0 : chunks[0]])
            load_chunk(nc.gpsimd, 1)
            if len(chunks) > 2:
                load_chunk(nc.sync, 2)
            for db in range(3, DBLK):
                nc.sync.dma_start(xts[0][:, db, :], xt[:, db, 0 : chunks[0]])
            for ci in range(3, len(chunks)):
                load_chunk(nc.sync, ci)

            # -------- main loop: L1 matmuls + relu per chunk; L2 of the
            # previous chunk slotted after L1 of the current one so the PE
            # never waits on the ACT epilogue.
            def emit_l2(pend, last=False):
                h_t, off, n = pend
                ps2 = pp2.tile([C, CHUNK], f32, name="ps2", tag="ps2")
                for hb in range(HBLK):
                    nc.tensor.matmul(
                        ps2[:, :n],
                        w_t[:, DBLK * H + hb * C : DBLK * H + (hb + 1) * C],
                        h_t[:, hb, :n],
                        start=(hb == 0),
                        stop=(hb == HBLK - 1),
                    )
                y_t = ypool.tile([C, n], f32, name="y_t", tag="y")
                nc.vector.tensor_scalar(
                    y_t[:, :n],
                    ps2[:, :n],
                    scalar1=b_t[:C, HBLK : HBLK + 1],
                    scalar2=None,
                    op0=add,
                )
                # final store goes HWDGE (scalar ring is idle by then and its
                # completion latency is ~1us shorter than SWDGE) so the
                # drain barrier fires sooner
                eng = nc.scalar if last else nc.gpsimd
                eng.dma_start(yt[:, off : off + n], y_t[:, :n])

            pending = None
            off = 0
            for ci, n in enumerate(chunks):
                x_t = xts[ci]
                h_t = hpool.tile([P, HBLK, n], bf16, name="h_t", tag="h")
                if ci == 0:
                    # db-outer: consume each arriving x d-block slice
                    # across all h-block accumulators immediately
                    pss = [
                        pp1.tile([P, n], f32, name="ps", tag="ps")
                        for _ in range(HBLK)
                    ]
                    for db in range(DBLK):
                        for hb in range(HBLK):
                            nc.tensor.matmul(
                                pss[hb][:, :n],
                                w_t[:, db * H + hb * P : db * H + (hb + 1) * P],
                                x_t[:, db, :n],
                                start=(db == 0),
                                stop=(db == DBLK - 1),
                            )
                    for hb in range(HBLK):
                        nc.scalar.activation(
                            h_t[:, hb, :n], pss[hb][:, :n], relu,
                            bias=b_t[:, hb : hb + 1],
                        )
                else:
                    for hb in range(HBLK):
                        ps = pp1.tile([P, n], f32, name="ps", tag="ps")
                        for db in range(DBLK):
                            nc.tensor.matmul(
                                ps[:, :n],
                                w_t[:, db * H + hb * P : db * H + (hb + 1) * P],
                                x_t[:, db, :n],
                                start=(db == 0),
                                stop=(db == DBLK - 1),
                            )
                        nc.scalar.activation(
                            h_t[:, hb, :n], ps[:, :n], relu,
                            bias=b_t[:, hb : hb + 1],
                        )
                        # slot L2 of the previous chunk right after this
                        # chunk's first h-block so its epilogue (bias add +
                        # store) overlaps the remaining L1 matmuls instead
                        # of dangling past the last one
                        if hb == 0 and pending is not None:
                            emit_l2(pending)
                            pending = None
                if pending is not None:
                    emit_l2(pending)
                    pending = None
                pending = (h_t, off, n)
                off += n
            emit_l2(pending, last=True)

    return _split_excess_waits(nc)


def kernel(x, W1, b1, W2, b2, question_types):
    global last_results
    x = np.ascontiguousarray(np.asarray(x, dtype=np.float32))
    W1 = np.asarray(W1, dtype=np.float32)
    b1 = np.asarray(b1, dtype=np.float32)
    W2 = np.asarray(W2, dtype=np.float32)
    b2 = np.asarray(b2, dtype=np.float32)
    qt = np.asarray(question_types)
    N = x.shape[0]

    idx = [np.nonzero(qt == e)[0] for e in range(E)]
    counts = [len(i) for i in idx]
    S = max(int(np.ceil(max(counts) / 16) * 16), 2 * LEAD + CHUNK)

    nc = _program_cache.get(S)
    if nc is None:
        nc = _build_program(S)
        _program_cache[S] = nc

    in_maps = []
    for e in range(E):
        cnt = counts[e]
        xp = np.zeros((S, D), np.float32)
        xp[:cnt] = x[idx[e]]
        xt = np.ascontiguousarray(
            xp.T.reshape(DBLK, P, S).transpose(1, 0, 2)
        ).astype(BF16NP)
        w1t = W1[e].reshape(DBLK, P, H).transpose(1, 0, 2).reshape(P, DBLK * H)
        w2t = W2[e].reshape(HBLK, P, C).transpose(1, 0, 2).reshape(P, HBLK * C)
        wt = np.ascontiguousarray(np.concatenate([w1t, w2t], axis=1)).astype(BF16NP)
        bt = np.zeros((P, HBLK + 1), np.float32)
        bt[:, :HBLK] = b1[e].reshape(HBLK, P).T
        bt[:C, HBLK] = b2[e]
        in_maps.append({"xt": xt, "wt": wt, "bt": bt})

    r = run_bass_kernel_spmd(nc, in_maps, list(range(NCORES)))
    last_results = r

    out = np.zeros((N, C), np.float32)
    for e in range(E):
        out[idx[e]] = r.results[e]["yt"][:, : counts[e]].T
    return out


# revision 17
# speedup vs baseline: 1.0384x; 1.0384x over previous
"""Routed MoE classifier head for Trainium2 (8 NeuronCores, SPMD).

The reference computes all 8 experts densely and selects; here each sample is
routed to exactly one expert.  On the host we gather samples by expert
(expert e -> core e), pad to a common S, and pre-transpose x so the
contraction dim D lands on SBUF partitions.  Each core runs a dense 2-layer
MLP (768 -> relu 384 -> 8) over its expert's samples:

  layer 1:  h^T = relu(W1^T x^T + b1)   as matmul(psum, lhsT=W1 [128,128],
            rhs=xT [128,n]) accumulated over 6 d-blocks per h-block
  layer 2:  y^T = W2^T h^T + b2

Matmul operands use bfloat16 (same 1 col/cycle PE stream rate as f32r, but
half the HBM traffic and no ap<256 stream penalty); PSUM accumulation stays
fp32.  All of x is DMA-preloaded into SBUF up front (it fits in bf16), and
the PE is pre-warmed with dummy matmuls so the HAM clock-gate ramp starts
during the DMA wait.  Output y^T [8, S] is scattered back on the host.
"""

import numpy as np
import ml_dtypes

import concourse.bass as bass
import concourse.mybir as mybir
from concourse.tile import TileContext
from concourse.bass_utils import run_bass_kernel_spmd

P = 128
D = 768
H = 384
C = 8
E = 8
NCORES = 8
DBLK = D // P  # 6
HBLK = H // P  # 3
CHUNK = 512  # compute chunk (one PSUM bank of fp32 — ISA max matmul width)
LEAD = 512  # first chunk (quick start while DMA streams)

BF16NP = ml_dtypes.bfloat16

_program_cache = {}
last_results = None  # BassKernelResults of the most recent run (for test harness)


def _split_excess_waits(nc, max_waits=1):
    """The walrus build in this container only encodes one sem-wait per
    instruction; hoist extra waits onto NOPs inserted just before."""
    for blk in nc.main_func.blocks:
        insts = blk.instructions
        i = 0
        while i < len(insts):
            inst = insts[i]
            si = getattr(inst, "sync_info", None)
            if si is not None and si.on_wait and len(si.on_wait) > max_waits:
                waits = list(si.on_wait)
                extra, keep = waits[:-max_waits], waits[-max_waits:]
                nops = []
                for j in range(0, len(extra), max_waits):
                    nops.append(
                        mybir.InstNoOp(
                            name=f"{inst.name}-wsplit{j}",
                            engine=inst.engine,
                            bass_nofuse=True,
                            sync_info=mybir.SyncInfo(
                                on_wait=extra[j : j + max_waits], on_update=[]
                            ),
                        )
                    )
                inst.sync_info = mybir.SyncInfo(on_wait=keep, on_update=si.on_update)
                for k, nop in enumerate(nops):
                    nc.register_instruction(nop, overwrite=True)
                    insts.insert(i + k, nop)
                i += len(nops)
            i += 1
    return nc


def _chunks_of(S):
    """Chunk widths covering S: CHUNK-wide, then tail."""
    chunks = []
    rem = S
    while rem >= CHUNK:
        chunks.append(CHUNK)
        rem -= CHUNK
    if rem:
        chunks.append(rem)
    return chunks


def _build_program(S):
    f32 = mybir.dt.float32
    bf16 = mybir.dt.bfloat16
    relu = mybir.ActivationFunctionType.Relu
    add = mybir.AluOpType.add

    chunks = _chunks_of(S)
    nch = len(chunks)

    nc = bass.Bass(enable_partition_id=False)
    # chunk-major, per-partition-contiguous x layout: one chunk DMA is 128
    # rows x 6KB (128 descriptors) instead of 768 x 1KB — HWDGE dispatch
    # time scales with descriptor count and was the startup bottleneck
    xt = nc.dram_tensor("xt", [nch, P, DBLK, CHUNK], bf16, kind="ExternalInput")
    # w1 (6*384 cols) and w2 (3*8 cols) packed on the same 128 partitions
    wt = nc.dram_tensor("wt", [P, DBLK * H + HBLK * C], bf16, kind="ExternalInput")
    # b1 (3 cols, per h-block) and b2 (1 col, rows 0..7) packed
    bt = nc.dram_tensor("bt", [P, HBLK + 1], f32, kind="ExternalInput")
    yt = nc.dram_tensor("yt", [C, S], f32, kind="ExternalOutput")

    with TileContext(nc) as tc:
        with (
            tc.tile_pool(name="const", bufs=1) as cpool,
            tc.tile_pool(name="xin", bufs=1) as xpool,
            tc.tile_pool(name="hbuf", bufs=3) as hpool,
            tc.tile_pool(name="yout", bufs=2) as ypool,
            tc.tile_pool(name="psum1", bufs=6, space="PSUM") as pp1,
            tc.tile_pool(name="psum2", bufs=2, space="PSUM") as pp2,
        ):
            # -------- startup: fan DMA dispatches across queues, pre-warm PE
            # PE clock ramp (HAM) needs ~3.4us of activity before it doubles
            # the clock; burn the DMA-wait window on dummy matmuls over a
            # zeroed scratch tile so real matmuls start closer to full speed.
            scratch = cpool.tile([P, 512], bf16)
            nc.gpsimd.memset(scratch[:], 0.0)
            # two fillers exactly cover the PE-idle window before the first
            # x slice lands; a third would push real work out instead
            for _ in range(2):
                pw = pp1.tile([P, LEAD], f32, name="ps", tag="ps")
                nc.tensor.matmul(
                    pw[:, :LEAD], scratch[:, :P], scratch[:, :LEAD],
                    start=True, stop=True,
                )

            # weights on the Scalar HWDGE ring in 4 pieces so each d-block's
            # weights land just ahead of the matmuls that need them
            w_t = cpool.tile([P, DBLK * H + HBLK * C], bf16)
            nc.scalar.dma_start(w_t[:, 0:H], wt[:, 0:H])
            nc.scalar.dma_start(w_t[:, H : 2 * H], wt[:, H : 2 * H])
            nc.scalar.dma_start(w_t[:, 2 * H : 4 * H], wt[:, 2 * H : 4 * H])
            nc.scalar.dma_start(w_t[:, 4 * H :], wt[:, 4 * H :])

            b_t = cpool.tile([P, HBLK + 1], f32)
            nc.gpsimd.dma_start(b_t[:], bt[:])

            # Warm the ACT table during the startup DMA window so the
            # first real relu doesn't pay the ~1.3us table load.
            warm = cpool.tile([P, 1], f32)
            nc.vector.memset(warm[:], 0.0)
            nc.scalar.activation(warm[:], warm[:], relu, bias=0.0)

            # ALL of x, dispatched up front on the Sync HWDGE ring (12.9MB
            # bf16 fits in SBUF; DMA at ~330GB/s finishes well ahead of the
            # ~74us PE stream).  Chunk 0 goes per-d-block so the very first
            # matmul only waits on a 128KB slice.
            # every chunk tile is a full [P, DBLK, CHUNK] (tail is
            # zero-padded in dram; compute only reads its first `n` columns)
            xts = [
                xpool.tile([P, DBLK, CHUNK], bf16, name=f"x{ci}", tag=f"x{ci}")
                for ci in range(nch)
            ]

            def load_chunk(ci):
                nc.sync.dma_start(xts[ci][:, :, :], xt[ci, :, :, :])

            # all x on the sync HWDGE ring; chunk 0 per-d-block so the first
            # matmul waits only on a 128KB slice, chunk 1 wedged in after
            # chunk 0's fourth d-block (FIFO transfer order matches the
            # consumption order of the cold-clock first two chunks)
            for db in range(4):
                nc.sync.dma_start(xts[0][:, db, :], xt[0, :, db, :])
            load_chunk(1)
            for db in range(4, DBLK):
                nc.sync.dma_start(xts[0][:, db, :], xt[0, :, db, :])
            for ci in range(2, nch):
                load_chunk(ci)

            # -------- main loop: L1 matmuls + relu per chunk; L2 of the
            # previous chunk slotted after L1 of the current one so the PE
            # never waits on the ACT epilogue.
            def emit_l2(pend, last=False):
                h_t, off, n = pend
                ps2 = pp2.tile([C, CHUNK], f32, name="ps2", tag="ps2")
                for hb in range(HBLK):
                    nc.tensor.matmul(
                        ps2[:, :n],
                        w_t[:, DBLK * H + hb * C : DBLK * H + (hb + 1) * C],
                        h_t[:, hb, :n],
                        start=(hb == 0),
                        stop=(hb == HBLK - 1),
                    )
                y_t = ypool.tile([C, n], f32, name="y_t", tag="y")
                nc.vector.tensor_scalar(
                    y_t[:, :n],
                    ps2[:, :n],
                    scalar1=b_t[:C, HBLK : HBLK + 1],
                    scalar2=None,
                    op0=add,
                )
                # final store goes HWDGE (sync ring is idle by then and its
                # completion latency is ~1us shorter than SWDGE) so the
                # drain barrier fires sooner
                eng = nc.sync if last else nc.gpsimd
                eng.dma_start(yt[:, off : off + n], y_t[:, :n])

            pending = None
            off = 0
            for ci, n in enumerate(chunks):
                x_t = xts[ci]
                h_t = hpool.tile([P, HBLK, n], bf16, name="h_t", tag="h")
                if ci == 0:
                    # db-outer: consume each arriving x d-block slice
                    # across all h-block accumulators immediately
                    pss = [
                        pp1.tile([P, n], f32, name="ps", tag="ps")
                        for _ in range(HBLK)
                    ]
                    for db in range(DBLK):
                        for hb in range(HBLK):
                            nc.tensor.matmul(
                                pss[hb][:, :n],
                                w_t[:, db * H + hb * P : db * H + (hb + 1) * P],
                                x_t[:, db, :n],
                                start=(db == 0),
                                stop=(db == DBLK - 1),
                            )
                    for hb in range(HBLK):
                        nc.scalar.activation(
                            h_t[:, hb, :n], pss[hb][:, :n], relu,
                            bias=b_t[:, hb : hb + 1],
                        )
                else:
                    for hb in range(HBLK):
                        ps = pp1.tile([P, n], f32, name="ps", tag="ps")
                        for db in range(DBLK):
                            nc.tensor.matmul(
                                ps[:, :n],
                                w_t[:, db * H + hb * P : db * H + (hb + 1) * P],
                                x_t[:, db, :n],
                                start=(db == 0),
                                stop=(db == DBLK - 1),
                            )
                        nc.scalar.activation(
                            h_t[:, hb, :n], ps[:, :n], relu,
                            bias=b_t[:, hb : hb + 1],
                        )
                        # slot L2 of the previous chunk right after this
                        # chunk's first h-block so its epilogue (bias add +
                        # store) overlaps the remaining L1 matmuls instead
                        # of dangling past the last one
                        if hb == 0 and pending is not None:
                            emit_l2(pending)
                            pending = None
                if pending is not None:
                    emit_l2(pending)
                    pending = None
                pending = (h_t, off, n)
                off += n
            emit_l2(pending, last=True)

    return _split_excess_waits(nc)


def kernel(x, W1, b1, W2, b2, question_types):
    global last_results
    x = np.ascontiguousarray(np.asarray(x, dtype=np.float32))
    W1 = np.asarray(W1, dtype=np.float32)
    b1 = np.asarray(b1, dtype=np.float32)
    W2 = np.asarray(W2, dtype=np.float32)
    b2 = np.asarray(b2, dtype=np.float32)
    qt = np.asarray(question_types)
    N = x.shape[0]

    idx = [np.nonzero(qt == e)[0] for e in range(E)]
    counts = [len(i) for i in idx]
    S = max(int(np.ceil(max(counts) / 16) * 16), 2 * LEAD + CHUNK)

    nc = _program_cache.get(S)
    if nc is None:
        nc = _build_program(S)
        _program_cache[S] = nc

    nch = len(_chunks_of(S))
    Sp = nch * CHUNK
    in_maps = []
    for e in range(E):
        cnt = counts[e]
        xp = np.zeros((Sp, D), np.float32)
        xp[:cnt] = x[idx[e]]
        # [nch, P, DBLK, CHUNK]: chunk-major with each chunk's data
        # contiguous per partition (matches the dram layout the program
        # declares, so one chunk DMA is 128 x 6KB rows)
        xt = np.ascontiguousarray(
            xp.T.reshape(DBLK, P, nch, CHUNK).transpose(2, 1, 0, 3)
        ).astype(BF16NP)
        w1t = W1[e].reshape(DBLK, P, H).transpose(1, 0, 2).reshape(P, DBLK * H)
        w2t = W2[e].reshape(HBLK, P, C).transpose(1, 0, 2).reshape(P, HBLK * C)
        wt = np.ascontiguousarray(np.concatenate([w1t, w2t], axis=1)).astype(BF16NP)
        bt = np.zeros((P, HBLK + 1), np.float32)
        bt[:, :HBLK] = b1[e].reshape(HBLK, P).T
        bt[:C, HBLK] = b2[e]
        in_maps.append({"xt": xt, "wt": wt, "bt": bt})

    r = run_bass_kernel_spmd(nc, in_maps, list(range(NCORES)))
    last_results = r

    out = np.zeros((N, C), np.float32)
    for e in range(E):
        out[idx[e]] = r.results[e]["yt"][:, : counts[e]].T
    return out


# revision 20
# speedup vs baseline: 1.0431x; 1.0046x over previous
"""Routed MoE classifier head for Trainium2 (8 NeuronCores, SPMD).

The reference computes all 8 experts densely and selects; here each sample is
routed to exactly one expert.  On the host we gather samples by expert
(expert e -> core e), pad to a common S, and pre-transpose x so the
contraction dim D lands on SBUF partitions.  Each core runs a dense 2-layer
MLP (768 -> relu 384 -> 8) over its expert's samples:

  layer 1:  h^T = relu(W1^T x^T + b1)   as matmul(psum, lhsT=W1 [128,128],
            rhs=xT [128,n]) accumulated over 6 d-blocks per h-block
  layer 2:  y^T = W2^T h^T + b2

Matmul operands use bfloat16 (same 1 col/cycle PE stream rate as f32r, but
half the HBM traffic and no ap<256 stream penalty); PSUM accumulation stays
fp32.  All of x is DMA-preloaded into SBUF up front (it fits in bf16), and
the PE is pre-warmed with dummy matmuls so the HAM clock-gate ramp starts
during the DMA wait.  Output y^T [8, S] is scattered back on the host.
"""

import numpy as np
import ml_dtypes

import concourse.bass as bass
import concourse.mybir as mybir
from concourse.tile import TileContext
from concourse.bass_utils import run_bass_kernel_spmd

P = 128
D = 768
H = 384
C = 8
E = 8
NCORES = 8
DBLK = D // P  # 6
HBLK = H // P  # 3
CHUNK = 512  # compute chunk (one PSUM bank of fp32 — ISA max matmul width)
LEAD = 512  # first chunk (quick start while DMA streams)

BF16NP = ml_dtypes.bfloat16

_program_cache = {}
last_results = None  # BassKernelResults of the most recent run (for test harness)


def _split_excess_waits(nc, max_waits=1):
    """The walrus build in this container only encodes one sem-wait per
    instruction; hoist extra waits onto NOPs inserted just before."""
    for blk in nc.main_func.blocks:
        insts = blk.instructions
        i = 0
        while i < len(insts):
            inst = insts[i]
            si = getattr(inst, "sync_info", None)
            if si is not None and si.on_wait and len(si.on_wait) > max_waits:
                waits = list(si.on_wait)
                extra, keep = waits[:-max_waits], waits[-max_waits:]
                nops = []
                for j in range(0, len(extra), max_waits):
                    nops.append(
                        mybir.InstNoOp(
                            name=f"{inst.name}-wsplit{j}",
                            engine=inst.engine,
                            bass_nofuse=True,
                            sync_info=mybir.SyncInfo(
                                on_wait=extra[j : j + max_waits], on_update=[]
                            ),
                        )
                    )
                inst.sync_info = mybir.SyncInfo(on_wait=keep, on_update=si.on_update)
                for k, nop in enumerate(nops):
                    nc.register_instruction(nop, overwrite=True)
                    insts.insert(i + k, nop)
                i += len(nops)
            i += 1
    return nc


def _chunks_of(S):
    """Chunk widths covering S: CHUNK-wide, then tail."""
    chunks = []
    rem = S
    while rem >= CHUNK:
        chunks.append(CHUNK)
        rem -= CHUNK
    if rem:
        chunks.append(rem)
    return chunks


def _build_program(S):
    f32 = mybir.dt.float32
    bf16 = mybir.dt.bfloat16
    relu = mybir.ActivationFunctionType.Relu
    add = mybir.AluOpType.add

    chunks = _chunks_of(S)
    nch = len(chunks)

    nc = bass.Bass(enable_partition_id=False)
    # chunk-major, per-partition-contiguous x layout: one chunk DMA is 128
    # rows x 6KB (128 descriptors) instead of 768 x 1KB — HWDGE dispatch
    # time scales with descriptor count and was the startup bottleneck
    xt = nc.dram_tensor("xt", [nch, P, DBLK, CHUNK], bf16, kind="ExternalInput")
    # w1 (6*384 cols) and w2 (3*8 cols) packed on the same 128 partitions
    wt = nc.dram_tensor("wt", [P, DBLK * H + HBLK * C], bf16, kind="ExternalInput")
    # b1 (3 cols, per h-block) and b2 (1 col, rows 0..7) packed
    bt = nc.dram_tensor("bt", [P, HBLK + 1], f32, kind="ExternalInput")
    yt = nc.dram_tensor("yt", [C, S], f32, kind="ExternalOutput")

    with TileContext(nc) as tc:
        with (
            tc.tile_pool(name="const", bufs=1) as cpool,
            tc.tile_pool(name="xin", bufs=1) as xpool,
            tc.tile_pool(name="hbuf", bufs=3) as hpool,
            tc.tile_pool(name="yout", bufs=2) as ypool,
            tc.tile_pool(name="psum1", bufs=6, space="PSUM") as pp1,
            tc.tile_pool(name="psum2", bufs=2, space="PSUM") as pp2,
        ):
            # -------- startup: fan DMA dispatches across queues, pre-warm PE
            # PE clock ramp (HAM) needs ~3.4us of activity before it doubles
            # the clock; burn the DMA-wait window on dummy matmuls over a
            # zeroed scratch tile so real matmuls start closer to full speed.
            scratch = cpool.tile([P, 512], bf16)
            nc.gpsimd.memset(scratch[:], 0.0)
            # three fillers cover the ~1.9us PE-idle window before the first
            # x slice lands; more would push real work out instead
            for _ in range(3):
                pw = pp1.tile([P, LEAD], f32, name="ps", tag="ps")
                nc.tensor.matmul(
                    pw[:, :LEAD], scratch[:, :P], scratch[:, :LEAD],
                    start=True, stop=True,
                )

            # weights on the Scalar HWDGE ring, one piece per d-block (w2
            # rides with the last) so each d-block's weights land just ahead
            # of the matmuls that need them
            w_t = cpool.tile([P, DBLK * H + HBLK * C], bf16)
            for db in range(DBLK):
                lo = db * H
                hi = (db + 1) * H if db < DBLK - 1 else DBLK * H + HBLK * C
                nc.scalar.dma_start(w_t[:, lo:hi], wt[:, lo:hi])

            b_t = cpool.tile([P, HBLK + 1], f32)
            nc.gpsimd.dma_start(b_t[:], bt[:])

            # Warm the ACT table during the startup DMA window so the
            # first real relu doesn't pay the ~1.3us table load.
            warm = cpool.tile([P, 1], f32)
            nc.vector.memset(warm[:], 0.0)
            nc.scalar.activation(warm[:], warm[:], relu, bias=0.0)

            # ALL of x, dispatched up front on the Sync HWDGE ring (12.9MB
            # bf16 fits in SBUF; DMA at ~330GB/s finishes well ahead of the
            # ~74us PE stream).  Chunk 0 goes per-d-block so the very first
            # matmul only waits on a 128KB slice.
            # every chunk tile is a full [P, DBLK, CHUNK] (tail is
            # zero-padded in dram; compute only reads its first `n` columns)
            xts = [
                xpool.tile([P, DBLK, CHUNK], bf16, name=f"x{ci}", tag=f"x{ci}")
                for ci in range(nch)
            ]

            def load_chunk(ci):
                nc.sync.dma_start(xts[ci][:, :, :], xt[ci, :, :, :])

            # all x on the sync HWDGE ring; chunk 0 per-d-block so the first
            # matmul waits only on a 128KB slice, chunk 1 wedged in after
            # chunk 0's fourth d-block (FIFO transfer order matches the
            # consumption order of the cold-clock first two chunks)
            for db in range(4):
                nc.sync.dma_start(xts[0][:, db, :], xt[0, :, db, :])
            load_chunk(1)
            for db in range(4, DBLK):
                nc.sync.dma_start(xts[0][:, db, :], xt[0, :, db, :])
            for ci in range(2, nch):
                load_chunk(ci)

            # -------- main loop: L1 matmuls + relu per chunk; L2 of the
            # previous chunk slotted after L1 of the current one so the PE
            # never waits on the ACT epilogue.
            def emit_l2(pend, last=False):
                h_t, off, n = pend
                ps2 = pp2.tile([C, CHUNK], f32, name="ps2", tag="ps2")
                for hb in range(HBLK):
                    nc.tensor.matmul(
                        ps2[:, :n],
                        w_t[:, DBLK * H + hb * C : DBLK * H + (hb + 1) * C],
                        h_t[:, hb, :n],
                        start=(hb == 0),
                        stop=(hb == HBLK - 1),
                    )
                y_t = ypool.tile([C, n], f32, name="y_t", tag="y")
                nc.vector.tensor_scalar(
                    y_t[:, :n],
                    ps2[:, :n],
                    scalar1=b_t[:C, HBLK : HBLK + 1],
                    scalar2=None,
                    op0=add,
                )
                # final store goes HWDGE (sync ring is idle by then and its
                # completion latency is ~1us shorter than SWDGE) so the
                # drain barrier fires sooner
                eng = nc.sync if last else nc.gpsimd
                eng.dma_start(yt[:, off : off + n], y_t[:, :n])

            pending = None
            off = 0
            for ci, n in enumerate(chunks):
                x_t = xts[ci]
                h_t = hpool.tile([P, HBLK, n], bf16, name="h_t", tag="h")
                if ci == 0:
                    # db-outer: consume each arriving x d-block slice
                    # across all h-block accumulators immediately
                    pss = [
                        pp1.tile([P, n], f32, name="ps", tag="ps")
                        for _ in range(HBLK)
                    ]
                    for db in range(DBLK):
                        for hb in range(HBLK):
                            nc.tensor.matmul(
                                pss[hb][:, :n],
                                w_t[:, db * H + hb * P : db * H + (hb + 1) * P],
                                x_t[:, db, :n],
                                start=(db == 0),
                                stop=(db == DBLK - 1),
                            )
                    for hb in range(HBLK):
                        nc.scalar.activation(
                            h_t[:, hb, :n], pss[hb][:, :n], relu,
                            bias=b_t[:, hb : hb + 1],
                        )
                else:
                    for hb in range(HBLK):
                        ps = pp1.tile([P, n], f32, name="ps", tag="ps")
                        for db in range(DBLK):
                            nc.tensor.matmul(
                                ps[:, :n],
                                w_t[:, db * H + hb * P : db * H + (hb + 1) * P],
                                x_t[:, db, :n],
                                start=(db == 0),
                                stop=(db == DBLK - 1),
                            )
                        nc.scalar.activation(
                            h_t[:, hb, :n], ps[:, :n], relu,
                            bias=b_t[:, hb : hb + 1],
                        )
                        # slot L2 of the previous chunk after this chunk's
                        # second h-block: late enough that the previous
                        # chunk's relu epilogue (3 serialized ACTs) has
                        # finished, early enough that its bias add + store
                        # overlap the remaining L1 matmuls
                        if hb == 1 and pending is not None:
                            emit_l2(pending)
                            pending = None
                if pending is not None:
                    emit_l2(pending)
                    pending = None
                pending = (h_t, off, n)
                off += n
            emit_l2(pending, last=True)

    return _split_excess_waits(nc)


def kernel(x, W1, b1, W2, b2, question_types):
    global last_results
    x = np.ascontiguousarray(np.asarray(x, dtype=np.float32))
    W1 = np.asarray(W1, dtype=np.float32)
    b1 = np.asarray(b1, dtype=np.float32)
    W2 = np.asarray(W2, dtype=np.float32)
    b2 = np.asarray(b2, dtype=np.float32)
    qt = np.asarray(question_types)
    N = x.shape[0]

    idx = [np.nonzero(qt == e)[0] for e in range(E)]
    counts = [len(i) for i in idx]
    S = max(int(np.ceil(max(counts) / 16) * 16), 2 * LEAD + CHUNK)

    nc = _program_cache.get(S)
    if nc is None:
        nc = _build_program(S)
        _program_cache[S] = nc

    nch = len(_chunks_of(S))
    Sp = nch * CHUNK
    in_maps = []
    for e in range(E):
        cnt = counts[e]
        xp = np.zeros((Sp, D), np.float32)
        xp[:cnt] = x[idx[e]]
        # [nch, P, DBLK, CHUNK]: chunk-major with each chunk's data
        # contiguous per partition (matches the dram layout the program
        # declares, so one chunk DMA is 128 x 6KB rows)
        xt = np.ascontiguousarray(
            xp.T.reshape(DBLK, P, nch, CHUNK).transpose(2, 1, 0, 3)
        ).astype(BF16NP)
        w1t = W1[e].reshape(DBLK, P, H).transpose(1, 0, 2).reshape(P, DBLK * H)
        w2t = W2[e].reshape(HBLK, P, C).transpose(1, 0, 2).reshape(P, HBLK * C)
        wt = np.ascontiguousarray(np.concatenate([w1t, w2t], axis=1)).astype(BF16NP)
        bt = np.zeros((P, HBLK + 1), np.float32)
        bt[:, :HBLK] = b1[e].reshape(HBLK, P).T
        bt[:C, HBLK] = b2[e]
        in_maps.append({"xt": xt, "wt": wt, "bt": bt})

    r = run_bass_kernel_spmd(nc, in_maps, list(range(NCORES)))
    last_results = r

    out = np.zeros((N, C), np.float32)
    for e in range(E):
        out[idx[e]] = r.results[e]["yt"][:, : counts[e]].T
    return out
